# revision 1
# baseline (speedup 1.0000x reference)
"""Multi-head causal attention on 8 TRN2 NeuronCores.

Problem: x[4,2048,1024] @ Wqkv.T -> 16-head causal attention -> @ Wout.T.

Sharding: core c handles batch b=c//2, head-group g=c%2 (8 heads of 64).
Each core computes qkv for its (batch, head-group) slice, causal attention,
and a partial out-projection over its 512 columns of Wout's input dim.
Host sums the two partials per batch (the all-reduce of the hint).

Per-core layouts (host pre-transposes so every matmul contraction dim lands
on SBUF partitions):
  xT   [1024 d, 2048 t]      wqkT [1024 d, 1024 (q|k)e]
  wvT  [1024 d,  512 e]      woT  [ 512 e, 1024 f]
All matmuls run fp32r (1 cycle/row at N>=256 vs 4 for fp32; ~1e-4 rel err).

Emission is software-pipelined to keep the PE dense (HAM stays at 2.4GHz):
the QKV-production matmul groups for t-chunk tc+1 are interleaved into the
attention phase of chunk tc as PE filler between head pairs; S^T for jb+1
is emitted before AV of jb so the PE never waits on the ACT exp.
"""

import sys

sys.path.insert(0, "/opt/trn_rl_repo")

import numpy as np

B, T, D, H = 4, 2048, 1024, 16
E = 512  # per-core head width (8 heads x 64)
ND = 8  # d chunks of 128
NTC = 4  # t chunks of 512
SCALE = 0.125  # 1/sqrt(64)

_NC_CACHE = {}


def build():
    if "nc" in _NC_CACHE:
        return _NC_CACHE["nc"]
    import concourse.bacc as bacc
    import concourse.mybir as mybir
    import concourse.tile as tile

    F32 = mybir.dt.float32
    F32R = mybir.dt.float32r
    EXP = mybir.ActivationFunctionType.Exp

    nc = bacc.Bacc("TRN2", target_bir_lowering=False, debug=False, num_devices=8)
    xT = nc.declare_dram_parameter("xT", [D, T], F32R, isOutput=False)
    wqkT = nc.declare_dram_parameter("wqkT", [D, 2 * E], F32R, isOutput=False)
    wvT = nc.declare_dram_parameter("wvT", [D, E], F32R, isOutput=False)
    woT = nc.declare_dram_parameter("woT", [E, D], F32R, isOutput=False)
    z = nc.declare_dram_parameter("z", [T, D], F32, isOutput=True)
    dbg = {}
    if _NC_CACHE.get("debug"):
        for nm, shp in [
            ("dqt", [128, 512]), ("dkt", [128, T]), ("dv", [128, 768]),
            ("dpt", [128, 1024]), ("dya", [96, 512]), ("dyb", [96, 512]),
            ("dysb", [128, 512]),
        ]:
            dbg[nm] = nc.declare_dram_parameter(nm, shp, F32, isOutput=True)

    with tile.TileContext(nc) as tc:
        with (
            tc.tile_pool(name="pw", bufs=8) as pw,
            tc.tile_pool(name="pwo", bufs=4) as pwo,
            tc.tile_pool(name="px", bufs=8) as px,
            tc.tile_pool(name="pkt", bufs=4) as pkt,
            tc.tile_pool(name="pqt", bufs=4) as pqt,
            tc.tile_pool(name="pv", bufs=16) as pv,
            tc.tile_pool(name="ppt", bufs=2) as ppt,
            tc.tile_pool(name="pr", bufs=2) as pr,
            tc.tile_pool(name="pysb", bufs=4) as pysb,
            tc.tile_pool(name="pzsb", bufs=1) as pzsb,
            tc.tile_pool(name="pone", bufs=1) as pone,
            tc.tile_pool(name="ps", bufs=2, space="PSUM") as ps,
            tc.tile_pool(name="pyd", bufs=2, space="PSUM") as pyd,
        ):
            # ---- weights
            wqk = []
            for dc in range(ND):
                t_ = pw.tile([128, 2 * E], F32R, tag="wqk")
                nc.sync.dma_start(t_[:], wqkT[dc * 128 : (dc + 1) * 128, :])
                wqk.append(t_)
            wv = []
            for dc in range(ND):
                t_ = pw.tile([128, E], F32R, tag="wv")
                nc.sync.dma_start(t_[:], wvT[dc * 128 : (dc + 1) * 128, :])
                wv.append(t_)
            wo = []
            for m in range(4):
                t_ = pwo.tile([128, D], F32R, tag="wo")
                nc.sync.dma_start(t_[:], woT[m * 128 : (m + 1) * 128, :])
                wo.append(t_)

            ones_f = pone.tile([128, 256], F32, tag="onef")
            nc.gpsimd.memset(ones_f[:], 1.0)

            # persistent K^T [e,t] tiles; pair m = heads 2m / 2m+1 at
            # partition rows 0:64 / 64:128
            kt = [
                pkt.tile([128, T], F32R, tag="kt", name=f"kt{i}")
                for i in range(4)
            ]
            vt = [None] * 16  # V tiles per 128-row t-block

            def emit_x_loads(tci):
                xs = []
                t0 = tci * 512
                for dc in range(ND):
                    t_ = px.tile([128, 512], F32R, tag="x", name="xs")
                    nc.sync.dma_start(
                        t_[:], xT[dc * 128 : (dc + 1) * 128, t0 : t0 + 512]
                    )
                    xs.append(t_)
                return xs

            def emit_qk_group(xs, m, tci):
                """m 0..3: Q chunk -> returns qt tile; 4..7: K chunk."""
                acc = ps.tile([128, 1024], F32, tag="st", name="acc")
                acc = acc[:, 0:512]
                for dc in range(ND):
                    nc.tensor.matmul(
                        acc[:],
                        wqk[dc][:, m * 128 : (m + 1) * 128],
                        xs[dc][:],
                        start=(dc == 0),
                        stop=(dc == ND - 1),
                    )
                if m < 4:
                    t_ = pqt.tile([128, 512], F32R, tag="qt", name="qt")
                    nc.vector.tensor_copy(t_[:], acc[:])
                    return t_
                t0 = tci * 512
                nc.vector.tensor_copy(kt[m - 4][:, t0 : t0 + 512], acc[:])
                return None

            def emit_v_group(xs, tci, ts):
                """V tile layout: [V_h(64) | ones(32)] per head, so AV
                lhsT [.., 96] slices put Y at PSUM rows 0:64 and the
                denominator at 64:96."""
                jb = 4 * tci + ts
                acc = ps.tile([128, 1024], F32, tag="st", name="vacc")
                acc = acc[:, 0:512]
                for dc in range(ND):
                    nc.tensor.matmul(
                        acc[:],
                        xs[dc][:, ts * 128 : (ts + 1) * 128],
                        wv[dc][:],
                        start=(dc == 0),
                        stop=(dc == ND - 1),
                    )
                t_ = pv.tile([128, 768], F32R, tag="v", name="vt")
                t4 = t_[:].rearrange("p (hh c) -> p hh c", hh=8)
                a4 = acc[:].rearrange("p (hh c) -> p hh c", hh=8)
                nc.vector.tensor_copy(t4[:, :, 0:64], a4[:])
                o4 = ones_f[:].rearrange("p (hh c) -> p hh c", hh=8)
                nc.vector.tensor_copy(t4[:, :, 64:96], o4[:])
                vt[jb] = t_

            # ---- prologue: chunk 0 inputs + QKV production
            xs_cur = emit_x_loads(0)
            qt_cur = [emit_qk_group(xs_cur, m, 0) for m in range(4)]
            for m in range(4, 8):
                emit_qk_group(xs_cur, m, 0)
            for ts in range(4):
                emit_v_group(xs_cur, 0, ts)

            for tc_i in range(NTC):
                if dbg and tc_i == 0:
                    nc.sync.dma_start(dbg["dqt"][:], qt_cur[0][:].bitcast(F32))
                    nc.sync.dma_start(dbg["dv"][:], vt[0][:].bitcast(F32))

                if tc_i + 1 < NTC:
                    xs_next = emit_x_loads(tc_i + 1)
                    qt_next = [None] * 4
                else:
                    xs_next = None
                    qt_next = None

                # ---- attention for i-chunk ci = tc_i
                # single pair at a time; a filler queue of next-chunk QKV
                # groups keeps the PE gapless (HAM stays warm) while ACT
                # runs the exps
                ysb_list = [None] * 4
                njb = 4 * tc_i + 4

                def emit_s(m, qtm, jb):
                    st = ps.tile([128, 1024], F32, tag="st", name="st")
                    for h in range(2):
                        nc.tensor.matmul(
                            st[:, h * 512 : h * 512 + 512],
                            kt[m][
                                h * 64 : h * 64 + 64,
                                jb * 128 : (jb + 1) * 128,
                            ],
                            qtm[h * 64 : h * 64 + 64, :],
                            start=True,
                            stop=True,
                        )
                    return st

                for m in range(4):
                    qtm = qt_cur[m]
                    ya = pyd.tile([96, 512], F32, tag="ya")
                    yb = pyd.tile([96, 512], F32, tag="yb")
                    st_next = emit_s(m, qtm, 0)
                    for jb in range(njb):
                        st = st_next
                        pt = ppt.tile([128, 1024], F32R, tag="pt", name="pt")
                        nc.scalar.activation(pt[:], st[:], EXP, scale=SCALE)
                        if jb + 1 < njb:
                            st_next = emit_s(m, qtm, jb + 1)
                        if dbg and tc_i == 0 and m == 0 and jb == 0:
                            nc.sync.dma_start(dbg["dpt"][:], pt[:].bitcast(F32))
                        if jb >= 4 * tc_i:
                            r = jb - 4 * tc_i
                            for h in range(2):
                                half = pt[:, h * 512 : h * 512 + 512]
                                nc.gpsimd.affine_select(
                                    out=half,
                                    in_=half,
                                    compare_op=mybir.AluOpType.is_ge,
                                    fill=0.0,
                                    base=-128 * r,
                                    pattern=[[1, 512]],
                                    channel_multiplier=-1,
                                )
                        first, last = (jb == 0), (jb == njb - 1)
                        nc.tensor.matmul(
                            ya[:],
                            vt[jb][:, m * 192 : m * 192 + 96],
                            pt[:, 0:512],
                            start=first,
                            stop=last,
                        )
                        nc.tensor.matmul(
                            yb[:],
                            vt[jb][:, m * 192 + 96 : m * 192 + 192],
                            pt[:, 512:1024],
                            start=first,
                            stop=last,
                        )
                    if dbg and tc_i == 0 and m == 0:
                        dya_sb = pzsb.tile(
                            [128, 1024], F32, tag="zsb", name="dya_sb"
                        )
                        nc.vector.tensor_copy(dya_sb[0:96, 0:512], ya[:])
                        nc.sync.dma_start(dbg["dya"][:], dya_sb[0:96, 0:512])
                        dyb_sb = pzsb.tile(
                            [128, 1024], F32, tag="zsb", name="dyb_sb"
                        )
                        nc.vector.tensor_copy(dyb_sb[0:96, 0:512], yb[:])
                        nc.sync.dma_start(dbg["dyb"][:], dyb_sb[0:96, 0:512])

                    rca = pr.tile([128, 512], F32, tag="rca", bufs=1)
                    nc.vector.tensor_copy(rca[64:65, :], ya[64:65, :])
                    rcb = pr.tile([128, 512], F32, tag="rcb", bufs=1)
                    nc.vector.tensor_copy(rcb[64:65, :], yb[64:65, :])
                    rc0 = pr.tile([1, 1024], F32, tag="rc0", bufs=1)
                    nc.sync.dma_start(rc0[0:1, 0:512], rca[64:65, :])
                    nc.sync.dma_start(rc0[0:1, 512:1024], rcb[64:65, :])
                    nc.vector.reciprocal_approx_fast(
                        rca[0:1, :], rc0[0:1, 0:512]
                    )
                    nc.vector.reciprocal_approx_fast(
                        rcb[0:1, :], rc0[0:1, 512:1024]
                    )
                    rba = pr.tile([128, 512], F32, tag="rba", bufs=2)
                    nc.gpsimd.partition_broadcast(rba[0:64, :], rca[0:1, :])
                    rbb = pr.tile([128, 512], F32, tag="rbb", bufs=2)
                    nc.gpsimd.partition_broadcast(rbb[0:64, :], rcb[0:1, :])
                    ytmp = pr.tile([128, 512], F32R, tag="ytmp", bufs=1)
                    nc.vector.tensor_mul(
                        ytmp[0:64, :], yb[0:64, :], rbb[0:64, :]
                    )
                    ysb = pysb.tile([128, 512], F32R, tag="ysb", name="ysb")
                    nc.vector.tensor_mul(
                        ysb[0:64, :], ya[0:64, :], rba[0:64, :]
                    )
                    nc.sync.dma_start(ysb[64:128, :], ytmp[0:64, :])
                    if dbg and tc_i == 0 and m == 0:
                        nc.sync.dma_start(dbg["dysb"][:], ysb[:].bitcast(F32))
                    ysb_list[m] = ysb

                # next chunk's QKV production fills the last pair's
                # normalize-drain before the out-projection needs it
                if xs_next is not None:
                    for mm in range(4):
                        qt_next[mm] = emit_qk_group(xs_next, mm, tc_i + 1)
                        emit_qk_group(xs_next, mm + 4, tc_i + 1)
                    for ts in range(4):
                        emit_v_group(xs_next, tc_i + 1, ts)

                # ---- out-projection for i-chunk tc_i
                for ib in range(4):
                    for fh in range(2):
                        zp = ps.tile([128, 1024], F32, tag="st", name="zp")
                        zp = zp[:, 0:512]
                        for m in range(4):
                            nc.tensor.matmul(
                                zp[:],
                                ysb_list[m][:, ib * 128 : (ib + 1) * 128],
                                wo[m][:, fh * 512 : fh * 512 + 512],
                                start=(m == 0),
                                stop=(m == 3),
                            )
                        zsb = pzsb.tile([128, 512], F32, tag="zsb", bufs=2)
                        nc.vector.tensor_copy(zsb[:], zp[:])
                        row = (4 * tc_i + ib) * 128
                        nc.sync.dma_start(
                            z[row : row + 128, fh * 512 : fh * 512 + 512],
                            zsb[:],
                        )

                qt_cur = qt_next
                xs_cur = xs_next

            if dbg:
                nc.sync.dma_start(dbg["dkt"][:], kt[0][:].bitcast(F32))

    nc.finalize()
    _NC_CACHE["nc"] = nc
    return nc


def _in_maps(x, Wqkv, Wout):
    x = np.ascontiguousarray(np.asarray(x, dtype=np.float32))
    Wqkv = np.ascontiguousarray(np.asarray(Wqkv, dtype=np.float32))
    Wout = np.ascontiguousarray(np.asarray(Wout, dtype=np.float32))
    xTs = [np.ascontiguousarray(x[b].T) for b in range(B)]
    maps = []
    for c in range(8):
        b, g = divmod(c, 2)
        qrows = Wqkv[E * g : E * g + E]
        krows = Wqkv[D + E * g : D + E * g + E]
        vrows = Wqkv[2 * D + E * g : 2 * D + E * g + E]
        maps.append(
            {
                "xT": xTs[b],
                "wqkT": np.ascontiguousarray(
                    np.concatenate([qrows, krows], axis=0).T
                ),
                "wvT": np.ascontiguousarray(vrows.T),
                "woT": np.ascontiguousarray(Wout[:, E * g : E * g + E].T),
            }
        )
    return maps


def _run(x, Wqkv, Wout, trace=False):
    from concourse.bass_utils import run_bass_kernel_spmd

    nc = build()
    res = run_bass_kernel_spmd(
        nc, _in_maps(x, Wqkv, Wout), core_ids=list(range(8)), trace=trace
    )
    out = np.empty((B, T, D), dtype=np.float32)
    for b in range(B):
        out[b] = res.results[2 * b]["z"] + res.results[2 * b + 1]["z"]
    return out, res


def kernel(x, Wqkv, Wout):
    out, _ = _run(x, Wqkv, Wout, trace=False)
    return out



# revision 13
# speedup vs baseline: 1.0400x; 1.0400x over previous
"""Multi-head causal attention on 8 TRN2 NeuronCores.

Problem: x[4,2048,1024] @ Wqkv.T -> 16-head causal attention -> @ Wout.T.

Sharding: core c handles batch b=c//2, head-group g=c%2 (8 heads of 64).
Each core computes qkv for its (batch, head-group) slice, causal attention,
and a partial out-projection over its 512 columns of Wout's input dim.
Host sums the two partials per batch (the all-reduce of the hint).

Per-core layouts (host pre-transposes so every matmul contraction dim lands
on SBUF partitions):
  xT   [1024 d, 2048 t]      wqkT [1024 d, 1024 (q|k)e]
  wvT  [1024 d,  512 e]      woT  [ 512 e, 1024 f]
All matmuls run fp32r (1 cycle/col at N>=256).

Schedule: the attention inner loop is ACT(exp)-bound (~1.15us/pair-tile vs
~0.64us of PE work), so all other PE work is injected as filler between
attention pair-iterations to keep the PE dense and HAM warm:
  - chunk tc (tc<3) attention hosts the QKV production of chunk tc+1
  - chunk 2 also hosts chunk 0's out-projection; chunk 3 hosts chunks 1-2's
    (chunk 3 has no QKV filler), chunk 3's own runs at the tail
DMAs are ordered so the first QKV matmul's operands land first (wqk/x
interleaved, wv next, wo deferred to chunk 1).
"""

import sys

sys.path.insert(0, "/opt/trn_rl_repo")

import numpy as np

B, T, D, H = 4, 2048, 1024, 16
E = 512  # per-core head width (8 heads x 64)
ND = 8  # d chunks of 128
NTC = 4  # t chunks of 512
SCALE = 0.125  # 1/sqrt(64)

_NC_CACHE = {}


def build():
    if "nc" in _NC_CACHE:
        return _NC_CACHE["nc"]
    import concourse.bacc as bacc
    import concourse.mybir as mybir
    import concourse.tile as tile

    F32 = mybir.dt.float32
    F32R = mybir.dt.float32r
    EXP = mybir.ActivationFunctionType.Exp

    nc = bacc.Bacc("TRN2", target_bir_lowering=False, debug=False, num_devices=8)
    xT = nc.declare_dram_parameter("xT", [D, T], F32R, isOutput=False)
    wqkT = nc.declare_dram_parameter("wqkT", [D, 2 * E], F32R, isOutput=False)
    wvT = nc.declare_dram_parameter("wvT", [D, E], F32R, isOutput=False)
    woT = nc.declare_dram_parameter("woT", [E, D], F32R, isOutput=False)
    z = nc.declare_dram_parameter("z", [T, D], F32, isOutput=True)

    with tile.TileContext(nc) as tc:
        with (
            tc.tile_pool(name="pw", bufs=8) as pw,
            tc.tile_pool(name="pwo", bufs=4) as pwo,
            tc.tile_pool(name="px", bufs=8) as px,
            tc.tile_pool(name="pkt", bufs=4) as pkt,
            tc.tile_pool(name="pqt", bufs=6) as pqt,
            tc.tile_pool(name="pv", bufs=16) as pv,
            tc.tile_pool(name="ppt", bufs=2) as ppt,
            tc.tile_pool(name="pr", bufs=1) as pr,
            tc.tile_pool(name="pysb", bufs=11) as pysb,
            tc.tile_pool(name="pzsb", bufs=1) as pzsb,
            tc.tile_pool(name="pone", bufs=1) as pone,
            tc.tile_pool(name="pst", bufs=2, space="PSUM") as pst,
            tc.tile_pool(name="pacc", bufs=2, space="PSUM") as pacc,
            tc.tile_pool(name="pyd", bufs=1, space="PSUM") as pyd,
        ):
            # ---- chunk-0 inputs, ordered for earliest first matmul:
            # wqk[dc]/x[dc] pairs interleaved, wv after, wo deferred.
            wqk = [None] * ND
            xs0 = [None] * ND
            for dc in range(ND):
                w_ = pw.tile([128, 2 * E], F32R, tag="wqk", name="wqk")
                nc.sync.dma_start(w_[:], wqkT[dc * 128 : (dc + 1) * 128, :])
                wqk[dc] = w_
                t_ = px.tile([128, 512], F32R, tag="x", name="xs")
                nc.sync.dma_start(t_[:], xT[dc * 128 : (dc + 1) * 128, 0:512])
                xs0[dc] = t_
            wv = []
            for dc in range(ND):
                t_ = pw.tile([128, E], F32R, tag="wv", name="wv")
                nc.sync.dma_start(t_[:], wvT[dc * 128 : (dc + 1) * 128, :])
                wv.append(t_)
            wo = [None] * 4  # loaded at chunk 1

            ones_f = pone.tile([128, 64], F32, tag="onef")
            nc.gpsimd.memset(ones_f[:], 1.0)

            # persistent K^T [e,t] tiles; pair m = heads 2m / 2m+1 at
            # partition rows 0:64 / 64:128
            kt = [
                pkt.tile([128, T], F32R, tag="kt", name=f"kt{i}")
                for i in range(4)
            ]
            vt = [None] * 16  # V tiles per 128-row t-block

            def emit_x_loads(tci):
                xs = []
                t0 = tci * 512
                for dc in range(ND):
                    t_ = px.tile([128, 512], F32R, tag="x", name="xs")
                    nc.sync.dma_start(
                        t_[:], xT[dc * 128 : (dc + 1) * 128, t0 : t0 + 512]
                    )
                    xs.append(t_)
                return xs

            def emit_qk_group(xs, m, tci):
                """m 0..3: Q chunk -> returns qt tile; 4..7: K chunk."""
                acc = pacc.tile([128, 512], F32, tag="acc", name="acc")
                for dc in range(ND):
                    nc.tensor.matmul(
                        acc[:],
                        wqk[dc][:, m * 128 : (m + 1) * 128],
                        xs[dc][:],
                        start=(dc == 0),
                        stop=(dc == ND - 1),
                    )
                if m < 4:
                    t_ = pqt.tile([128, 512], F32R, tag="qt", name="qt")
                    nc.vector.tensor_copy(t_[:], acc[:])
                    return t_
                t0 = tci * 512
                nc.vector.tensor_copy(kt[m - 4][:, t0 : t0 + 512], acc[:])
                return None

            def emit_v_group(xs, tci, ts):
                """V tile layout: [V_h(64) | ones(8)] per head, so AV
                lhsT [.., 72] slices put Y at PSUM rows 0:64 and the
                denominator at 64:72."""
                jb = 4 * tci + ts
                acc = pacc.tile([128, 512], F32, tag="acc", name="vacc")
                for dc in range(ND):
                    nc.tensor.matmul(
                        acc[:],
                        xs[dc][:, ts * 128 : (ts + 1) * 128],
                        wv[dc][:],
                        start=(dc == 0),
                        stop=(dc == ND - 1),
                    )
                t_ = pv.tile([128, 576], F32R, tag="v", name="vt")
                t4 = t_[:].rearrange("p (hh c) -> p hh c", hh=8)
                a4 = acc[:].rearrange("p (hh c) -> p hh c", hh=8)
                nc.vector.tensor_copy(t4[:, :, 0:64], a4[:])
                o4 = ones_f[:].rearrange("p (hh c) -> p hh c", hh=8)
                nc.vector.tensor_copy(t4[:, :, 64:72], o4[:])
                vt[jb] = t_

            def emit_wo_loads():
                for m in range(4):
                    t_ = pwo.tile([128, D], F32R, tag="wo", name="wo")
                    nc.sync.dma_start(t_[:], woT[m * 128 : (m + 1) * 128, :])
                    wo[m] = t_

            ysb_all = [[None] * 4 for _ in range(NTC)]

            def emit_op_block(c, ib, fh):
                """Out-projection for chunk c, row-block ib, col-half fh."""
                zp = pacc.tile([128, 512], F32, tag="acc", name="zp")
                for m in range(4):
                    nc.tensor.matmul(
                        zp[:],
                        ysb_all[c][m][:, ib * 128 : (ib + 1) * 128],
                        wo[m][:, fh * 512 : fh * 512 + 512],
                        start=(m == 0),
                        stop=(m == 3),
                    )
                zsb = pzsb.tile([128, 512], F32, tag="zsb", name="zsb")
                nc.vector.tensor_copy(zsb[:], zp[:])
                row = (4 * c + ib) * 128
                nc.sync.dma_start(
                    z[row : row + 128, fh * 512 : fh * 512 + 512], zsb[:]
                )

            # ---- prologue: chunk 0 QKV production
            qt_cur = [emit_qk_group(xs0, m, 0) for m in range(4)]
            for m in range(4, 8):
                emit_qk_group(xs0, m, 0)
            for ts in range(4):
                emit_v_group(xs0, 0, ts)

            for tc_i in range(NTC):
                xs_next = emit_x_loads(tc_i + 1) if tc_i + 1 < NTC else None
                if tc_i == 1:
                    emit_wo_loads()
                qt_next = [None] * 4 if xs_next is not None else None

                # filler queue: closures emitting independent PE work,
                # injected between attention pair-iterations
                fillers = []
                if xs_next is not None:

                    def mk_qk(mm):
                        def f():
                            r = emit_qk_group(xs_next, mm, tc_i + 1)
                            if mm < 4:
                                qt_next[mm] = r

                        return f

                    def mk_v(ts):
                        return lambda: emit_v_group(xs_next, tc_i + 1, ts)

                    for mm in range(4):
                        fillers.append(mk_qk(mm))
                        fillers.append(mk_qk(mm + 4))
                        fillers.append(mk_v(mm))
                if tc_i == 2:
                    for ib in range(4):
                        for fh in range(2):
                            fillers.append(
                                lambda ib=ib, fh=fh: emit_op_block(0, ib, fh)
                            )
                if tc_i == 3:
                    for c in (1, 2):
                        for ib in range(4):
                            for fh in range(2):
                                fillers.append(
                                    lambda c=c, ib=ib, fh=fh: emit_op_block(
                                        c, ib, fh
                                    )
                                )

                njb = 4 * tc_i + 4
                niters = 4 * njb
                nfill = len(fillers)
                npopped = 0
                it_g = 0

                def emit_s(m, qtm, jb):
                    st = pst.tile([128, 1024], F32, tag="st", name="st")
                    for h in range(2):
                        nc.tensor.matmul(
                            st[:, h * 512 : h * 512 + 512],
                            kt[m][
                                h * 64 : h * 64 + 64,
                                jb * 128 : (jb + 1) * 128,
                            ],
                            qtm[h * 64 : h * 64 + 64, :],
                            start=True,
                            stop=True,
                        )
                    return st

                for m in range(4):
                    qtm = qt_cur[m]
                    ya = pyd.tile([72, 512], F32, tag="ya")
                    yb = pyd.tile([72, 512], F32, tag="yb")
                    st_next = emit_s(m, qtm, 0)
                    for jb in range(njb):
                        st = st_next
                        pt = ppt.tile([128, 1024], F32R, tag="pt", name="pt")
                        nc.scalar.activation(pt[:], st[:], EXP, scale=SCALE)
                        if jb + 1 < njb:
                            st_next = emit_s(m, qtm, jb + 1)
                        # PE filler while ACT runs the exp
                        it_g += 1
                        while fillers and npopped < it_g * nfill // niters:
                            fillers.pop(0)()
                            npopped += 1
                        if jb >= 4 * tc_i:
                            r = jb - 4 * tc_i
                            for h in range(2):
                                half = pt[:, h * 512 : h * 512 + 512]
                                nc.gpsimd.affine_select(
                                    out=half,
                                    in_=half,
                                    compare_op=mybir.AluOpType.is_ge,
                                    fill=0.0,
                                    base=-128 * r,
                                    pattern=[[1, 512]],
                                    channel_multiplier=-1,
                                )
                        first, last = (jb == 0), (jb == njb - 1)
                        nc.tensor.matmul(
                            ya[:],
                            vt[jb][:, m * 144 : m * 144 + 72],
                            pt[:, 0:512],
                            start=first,
                            stop=last,
                        )
                        nc.tensor.matmul(
                            yb[:],
                            vt[jb][:, m * 144 + 72 : m * 144 + 144],
                            pt[:, 512:1024],
                            start=first,
                            stop=last,
                        )

                    # normalize: 1/denominator broadcast along partitions,
                    # then scale Y halves into the ysb tile. recip and
                    # partition_broadcast require partition-0 sources on HW
                    # (AP partition offsets are ignored by their ucode), so
                    # vector-copy the denom rows 64 -> 0 first.
                    rci = pr.tile([128, 1024], F32, tag="rci", bufs=1)
                    nc.vector.tensor_copy(rci[0:1, 0:512], ya[64:65, :])
                    nc.vector.tensor_copy(rci[0:1, 512:1024], yb[64:65, :])
                    rco = pr.tile([128, 1024], F32, tag="rco", bufs=1)
                    nc.vector.reciprocal_approx_fast(rco[0:1, :], rci[0:1, :])
                    rb = pr.tile([128, 1024], F32, tag="rb", bufs=1)
                    nc.gpsimd.partition_broadcast(rb[0:64, :], rco[0:1, :])
                    ytmp = pr.tile([128, 512], F32R, tag="ytmp", bufs=1)
                    nc.vector.tensor_mul(
                        ytmp[0:64, :], yb[0:64, :], rb[0:64, 512:1024]
                    )
                    ysb = pysb.tile([128, 512], F32R, tag="ysb", name="ysb")
                    nc.vector.tensor_mul(
                        ysb[0:64, :], ya[0:64, :], rb[0:64, 0:512]
                    )
                    nc.sync.dma_start(ysb[64:128, :], ytmp[0:64, :])
                    ysb_all[tc_i][m] = ysb

                # drain any leftover fillers
                while fillers:
                    fillers.pop(0)()

                if tc_i == 3:
                    for ib in range(4):
                        for fh in range(2):
                            emit_op_block(3, ib, fh)

                qt_cur = qt_next

    nc.finalize()
    _NC_CACHE["nc"] = nc
    return nc


def _in_maps(x, Wqkv, Wout):
    x = np.ascontiguousarray(np.asarray(x, dtype=np.float32))
    Wqkv = np.ascontiguousarray(np.asarray(Wqkv, dtype=np.float32))
    Wout = np.ascontiguousarray(np.asarray(Wout, dtype=np.float32))
    xTs = [np.ascontiguousarray(x[b].T) for b in range(B)]
    maps = []
    for c in range(8):
        b, g = divmod(c, 2)
        qrows = Wqkv[E * g : E * g + E]
        krows = Wqkv[D + E * g : D + E * g + E]
        vrows = Wqkv[2 * D + E * g : 2 * D + E * g + E]
        maps.append(
            {
                "xT": xTs[b],
                "wqkT": np.ascontiguousarray(
                    np.concatenate([qrows, krows], axis=0).T
                ),
                "wvT": np.ascontiguousarray(vrows.T),
                "woT": np.ascontiguousarray(Wout[:, E * g : E * g + E].T),
            }
        )
    return maps


def _run(x, Wqkv, Wout, trace=False):
    from concourse.bass_utils import run_bass_kernel_spmd

    nc = build()
    res = run_bass_kernel_spmd(
        nc, _in_maps(x, Wqkv, Wout), core_ids=list(range(8)), trace=trace
    )
    out = np.empty((B, T, D), dtype=np.float32)
    for b in range(B):
        out[b] = res.results[2 * b]["z"] + res.results[2 * b + 1]["z"]
    return out, res


def kernel(x, Wqkv, Wout):
    out, _ = _run(x, Wqkv, Wout, trace=False)
    return out


# revision 19
# speedup vs baseline: 1.2615x; 1.2129x over previous
"""Multi-head causal attention on 8 TRN2 NeuronCores.

Problem: x[4,2048,1024] @ Wqkv.T -> 16-head causal attention -> @ Wout.T.

Sharding: core c handles batch b=c//2, head-group g=c%2 (8 heads of 64).
Each core computes qkv for its (batch, head-group) slice, causal attention,
and a partial out-projection over its 512 columns of Wout's input dim.
Host sums the two partials per batch (the all-reduce of the hint).

Per-core layouts (host pre-transposes so every matmul contraction dim lands
on SBUF partitions):
  xT   [1024 d, 2048 t]      wqkT [1024 d, 1024 (q|k)e]
  wvT  [1024 d,  512 e]      woT  [ 512 e, 1024 f]
All matmuls run fp32r (1 cycle/col at N>=256).

Schedule: the attention inner loop is ACT(exp)-bound (~1.15us/pair-tile vs
~0.64us of PE work), so all other PE work is injected as filler between
attention pair-iterations to keep the PE dense and HAM warm:
  - chunk tc (tc<3) attention hosts the QKV production of chunk tc+1
  - chunk 2 also hosts chunk 0's out-projection; chunk 3 hosts chunks 1-2's
    (chunk 3 has no QKV filler), chunk 3's own runs at the tail
DMAs are ordered so the first QKV matmul's operands land first (wqk/x
interleaved, wv next, wo deferred to chunk 1).
"""

import sys

sys.path.insert(0, "/opt/trn_rl_repo")

import numpy as np

B, T, D, H = 4, 2048, 1024, 16
E = 512  # per-core head width (8 heads x 64)
ND = 8  # d chunks of 128
NTC = 4  # t chunks of 512
SCALE = 0.125  # 1/sqrt(64)

_NC_CACHE = {}


def build():
    if "nc" in _NC_CACHE:
        return _NC_CACHE["nc"]
    import concourse.bacc as bacc
    import concourse.mybir as mybir
    import concourse.tile as tile

    F32 = mybir.dt.float32
    F32R = mybir.dt.float32r
    BF16 = mybir.dt.bfloat16
    EXP = mybir.ActivationFunctionType.Exp

    nc = bacc.Bacc("TRN2", target_bir_lowering=False, debug=False, num_devices=8)
    xT = nc.declare_dram_parameter("xT", [D, T], BF16, isOutput=False)
    wqkT = nc.declare_dram_parameter("wqkT", [D, 2 * E], BF16, isOutput=False)
    wvT = nc.declare_dram_parameter("wvT", [D, E], BF16, isOutput=False)
    woT = nc.declare_dram_parameter("woT", [E, D], F32R, isOutput=False)
    z = nc.declare_dram_parameter("z", [T, D], F32, isOutput=True)

    with tile.TileContext(nc) as tc:
        with (
            tc.tile_pool(name="pw", bufs=8) as pw,
            tc.tile_pool(name="pwo", bufs=4) as pwo,
            tc.tile_pool(name="px", bufs=8) as px,
            tc.tile_pool(name="pkt", bufs=4) as pkt,
            tc.tile_pool(name="pqt", bufs=8) as pqt,
            tc.tile_pool(name="pv", bufs=16) as pv,
            tc.tile_pool(name="ppt", bufs=2) as ppt,
            tc.tile_pool(name="pr", bufs=1) as pr,
            tc.tile_pool(name="pysb", bufs=12) as pysb,
            tc.tile_pool(name="pzsb", bufs=2) as pzsb,
            tc.tile_pool(name="pone", bufs=1) as pone,
            tc.tile_pool(name="pst", bufs=2, space="PSUM") as pst,
            tc.tile_pool(name="pacc", bufs=2, space="PSUM") as pacc,
            tc.tile_pool(name="pyd", bufs=1, space="PSUM") as pyd,
        ):
            # ---- chunk-0 inputs, ordered for earliest first matmul:
            # wqk[dc]/x[dc] pairs interleaved, wv after, wo deferred.
            wqk = [None] * ND
            xs0 = [None] * ND
            for dc in range(ND):
                w_ = pw.tile([128, 2 * E], BF16, tag="wqk", name="wqk")
                nc.sync.dma_start(w_[:], wqkT[dc * 128 : (dc + 1) * 128, :])
                wqk[dc] = w_
                t_ = px.tile([128, 512], BF16, tag="x", name="xs")
                nc.sync.dma_start(t_[:], xT[dc * 128 : (dc + 1) * 128, 0:512])
                xs0[dc] = t_
            wv = []
            for dc in range(ND):
                t_ = pw.tile([128, E], BF16, tag="wv", name="wv")
                nc.sync.dma_start(t_[:], wvT[dc * 128 : (dc + 1) * 128, :])
                wv.append(t_)
            wo = [None] * 4  # loaded at chunk 1

            ones_f = pone.tile([128, 64], F32, tag="onef")
            nc.gpsimd.memset(ones_f[:], 1.0)

            # persistent K^T [e,t] tiles; pair m = heads 2m / 2m+1 at
            # partition rows 0:64 / 64:128
            kt = [
                pkt.tile([128, T], BF16, tag="kt", name=f"kt{i}")
                for i in range(4)
            ]
            vt = [None] * 16  # V tiles per 128-row t-block

            def emit_x_loads(tci):
                xs = []
                t0 = tci * 512
                for dc in range(ND):
                    t_ = px.tile([128, 512], BF16, tag="x", name="xs")
                    nc.sync.dma_start(
                        t_[:], xT[dc * 128 : (dc + 1) * 128, t0 : t0 + 512]
                    )
                    xs.append(t_)
                return xs

            def emit_qk_group(xs, m, tci):
                """m 0..3: Q chunk -> returns qt tile; 4..7: K chunk."""
                acc = pacc.tile([128, 512], F32, tag="acc", name="acc")
                for dc in range(ND):
                    nc.tensor.matmul(
                        acc[:],
                        wqk[dc][:, m * 128 : (m + 1) * 128],
                        xs[dc][:],
                        start=(dc == 0),
                        stop=(dc == ND - 1),
                    )
                if m < 4:
                    t_ = pqt.tile([128, 512], BF16, tag="qt", name="qt")
                    nc.vector.tensor_copy(t_[:], acc[:])
                    return t_
                t0 = tci * 512
                nc.vector.tensor_copy(kt[m - 4][:, t0 : t0 + 512], acc[:])
                return None

            def emit_v_group(xs, tci, ts):
                """V tile layout: [V_h(64) | ones(8)] per head, so AV
                lhsT [.., 72] slices put Y at PSUM rows 0:64 and the
                denominator at 64:72."""
                jb = 4 * tci + ts
                acc = pacc.tile([128, 512], F32, tag="acc", name="vacc")
                for dc in range(ND):
                    nc.tensor.matmul(
                        acc[:],
                        xs[dc][:, ts * 128 : (ts + 1) * 128],
                        wv[dc][:],
                        start=(dc == 0),
                        stop=(dc == ND - 1),
                    )
                t_ = pv.tile([128, 576], F32R, tag="v", name="vt")
                t4 = t_[:].rearrange("p (hh c) -> p hh c", hh=8)
                a4 = acc[:].rearrange("p (hh c) -> p hh c", hh=8)
                nc.vector.tensor_copy(t4[:, :, 0:64], a4[:])
                o4 = ones_f[:].rearrange("p (hh c) -> p hh c", hh=8)
                nc.vector.tensor_copy(t4[:, :, 64:72], o4[:])
                vt[jb] = t_

            def emit_wo_loads():
                for m in range(4):
                    t_ = pwo.tile([128, D], F32R, tag="wo", name="wo")
                    nc.sync.dma_start(t_[:], woT[m * 128 : (m + 1) * 128, :])
                    wo[m] = t_

            ysb_all = [[None] * 4 for _ in range(NTC)]

            def emit_op_block(c, ib, fh):
                """Out-projection for chunk c, row-block ib, col-half fh."""
                zp = pacc.tile([128, 512], F32, tag="acc", name="zp")
                for m in range(4):
                    nc.tensor.matmul(
                        zp[:],
                        ysb_all[c][m][:, ib * 128 : (ib + 1) * 128],
                        wo[m][:, fh * 512 : fh * 512 + 512],
                        start=(m == 0),
                        stop=(m == 3),
                    )
                zsb = pzsb.tile([128, 512], F32, tag="zsb", name="zsb")
                nc.vector.tensor_copy(zsb[:], zp[:])
                row = (4 * c + ib) * 128
                nc.scalar.dma_start(
                    z[row : row + 128, fh * 512 : fh * 512 + 512], zsb[:]
                )

            # ---- prologue: chunk 0 QKV production
            qt_cur = [emit_qk_group(xs0, m, 0) for m in range(4)]
            for m in range(4, 8):
                emit_qk_group(xs0, m, 0)
            for ts in range(4):
                emit_v_group(xs0, 0, ts)

            for tc_i in range(NTC):
                xs_next = emit_x_loads(tc_i + 1) if tc_i + 1 < NTC else None
                if tc_i == 1:
                    emit_wo_loads()
                qt_next = [None] * 4 if xs_next is not None else None

                # filler queue: closures emitting independent PE work,
                # injected between attention pair-iterations
                fillers = []
                if xs_next is not None:

                    def mk_qk(mm):
                        def f():
                            r = emit_qk_group(xs_next, mm, tc_i + 1)
                            if mm < 4:
                                qt_next[mm] = r

                        return f

                    def mk_v(ts):
                        return lambda: emit_v_group(xs_next, tc_i + 1, ts)

                    for mm in range(4):
                        fillers.append(mk_qk(mm))
                        fillers.append(mk_qk(mm + 4))
                        fillers.append(mk_v(mm))
                if tc_i == 2:
                    for ib in range(4):
                        for fh in range(2):
                            fillers.append(
                                lambda ib=ib, fh=fh: emit_op_block(0, ib, fh)
                            )
                if tc_i == 3:
                    for c in (1, 2):
                        for ib in range(4):
                            for fh in range(2):
                                fillers.append(
                                    lambda c=c, ib=ib, fh=fh: emit_op_block(
                                        c, ib, fh
                                    )
                                )

                njb = 4 * tc_i + 4
                niters = 4 * njb
                nfill = len(fillers)
                npopped = 0
                it_g = 0
                # delay QKV fillers until their x-chunk DMAs have landed
                lead = 5 if xs_next is not None else 0

                def off_of(jb):
                    """Diagonal blocks only see queries >= 128*(jb-4*tc_i);
                    trim the streamed query range to the causal triangle."""
                    return 128 * (jb - 4 * tc_i) if jb >= 4 * tc_i else 0

                def emit_s(m, qtm, jb):
                    off = off_of(jb)
                    st = pst.tile([128, 1024], F32, tag="st", name="st")
                    for h in range(2):
                        nc.tensor.matmul(
                            st[:, h * 512 + off : h * 512 + 512],
                            kt[m][
                                h * 64 : h * 64 + 64,
                                jb * 128 : (jb + 1) * 128,
                            ],
                            qtm[h * 64 : h * 64 + 64, off:512],
                            start=True,
                            stop=True,
                        )
                    return st

                for m in range(4):
                    qtm = qt_cur[m]
                    ya = pyd.tile([72, 512], F32, tag="ya")
                    yb = pyd.tile([72, 512], F32, tag="yb")
                    st_next = emit_s(m, qtm, 0)
                    for jb in range(njb):
                        off = off_of(jb)
                        st = st_next
                        pt = ppt.tile([128, 1024], F32R, tag="pt", name="pt")
                        if off:
                            stv = st[:].rearrange("p (h q) -> p h q", h=2)
                            ptv = pt[:].rearrange("p (h q) -> p h q", h=2)
                            nc.scalar.activation(
                                ptv[:, :, off:512],
                                stv[:, :, off:512],
                                EXP,
                                scale=SCALE,
                            )
                        else:
                            nc.scalar.activation(pt[:], st[:], EXP, scale=SCALE)
                        if jb + 1 < njb:
                            st_next = emit_s(m, qtm, jb + 1)
                        # PE filler while ACT runs the exp
                        it_g += 1
                        adj = max(0, it_g - lead)
                        while fillers and npopped < adj * nfill // (
                            niters - lead
                        ):
                            fillers.pop(0)()
                            npopped += 1
                        if jb >= 4 * tc_i:
                            for h in range(2):
                                half = pt[:, h * 512 + off : h * 512 + 512]
                                nc.gpsimd.affine_select(
                                    out=half,
                                    in_=half,
                                    compare_op=mybir.AluOpType.is_ge,
                                    fill=0.0,
                                    base=0,
                                    pattern=[[1, 512 - off]],
                                    channel_multiplier=-1,
                                )
                        first, last = (jb == 0), (jb == njb - 1)
                        nc.tensor.matmul(
                            ya[:, off:512],
                            vt[jb][:, m * 144 : m * 144 + 72],
                            pt[:, off:512],
                            start=first,
                            stop=last,
                        )
                        nc.tensor.matmul(
                            yb[:, off:512],
                            vt[jb][:, m * 144 + 72 : m * 144 + 144],
                            pt[:, 512 + off : 1024],
                            start=first,
                            stop=last,
                        )

                    # normalize: 1/denominator broadcast along partitions,
                    # then scale Y halves into the ysb tile. recip and
                    # partition_broadcast require partition-0 sources on HW
                    # (AP partition offsets are ignored by their ucode), so
                    # vector-copy the denom rows 64 -> 0 first.
                    rci = pr.tile([128, 1024], F32, tag="rci", bufs=1)
                    nc.vector.tensor_copy(rci[0:1, 0:512], ya[64:65, :])
                    nc.vector.tensor_copy(rci[0:1, 512:1024], yb[64:65, :])
                    rco = pr.tile([128, 1024], F32, tag="rco", bufs=1)
                    nc.vector.reciprocal_approx_fast(rco[0:1, :], rci[0:1, :])
                    rb = pr.tile([128, 1024], F32, tag="rb", bufs=1)
                    nc.gpsimd.partition_broadcast(rb[0:64, :], rco[0:1, :])
                    ytmp = pr.tile([128, 512], F32R, tag="ytmp", bufs=1)
                    nc.vector.tensor_mul(
                        ytmp[0:64, :], yb[0:64, :], rb[0:64, 512:1024]
                    )
                    ysb = pysb.tile([128, 512], F32R, tag="ysb", name="ysb")
                    nc.vector.tensor_mul(
                        ysb[0:64, :], ya[0:64, :], rb[0:64, 0:512]
                    )
                    nc.scalar.dma_start(ysb[64:128, :], ytmp[0:64, :])
                    ysb_all[tc_i][m] = ysb

                # drain any leftover fillers
                while fillers:
                    fillers.pop(0)()

                if tc_i == 3:
                    for ib in range(4):
                        for fh in range(2):
                            emit_op_block(3, ib, fh)

                qt_cur = qt_next

    nc.finalize()
    _NC_CACHE["nc"] = nc
    return nc


def _in_maps(x, Wqkv, Wout):
    import ml_dtypes

    BF = ml_dtypes.bfloat16
    x = np.ascontiguousarray(np.asarray(x, dtype=np.float32))
    Wqkv = np.ascontiguousarray(np.asarray(Wqkv, dtype=np.float32))
    Wout = np.ascontiguousarray(np.asarray(Wout, dtype=np.float32))
    xTs = [np.ascontiguousarray(x[b].T.astype(BF)) for b in range(B)]
    maps = []
    for c in range(8):
        b, g = divmod(c, 2)
        qrows = Wqkv[E * g : E * g + E]
        krows = Wqkv[D + E * g : D + E * g + E]
        vrows = Wqkv[2 * D + E * g : 2 * D + E * g + E]
        maps.append(
            {
                "xT": xTs[b],
                "wqkT": np.ascontiguousarray(
                    np.concatenate([qrows, krows], axis=0).T.astype(BF)
                ),
                "wvT": np.ascontiguousarray(vrows.T.astype(BF)),
                "woT": np.ascontiguousarray(Wout[:, E * g : E * g + E].T),
            }
        )
    return maps


def _run(x, Wqkv, Wout, trace=False):
    from concourse.bass_utils import run_bass_kernel_spmd

    nc = build()
    res = run_bass_kernel_spmd(
        nc, _in_maps(x, Wqkv, Wout), core_ids=list(range(8)), trace=trace
    )
    out = np.empty((B, T, D), dtype=np.float32)
    for b in range(B):
        out[b] = res.results[2 * b]["z"] + res.results[2 * b + 1]["z"]
    return out, res


def kernel(x, Wqkv, Wout):
    out, _ = _run(x, Wqkv, Wout, trace=False)
    return out


# revision 20
# speedup vs baseline: 1.3509x; 1.0709x over previous
"""Multi-head causal attention on 8 TRN2 NeuronCores.

Problem: x[4,2048,1024] @ Wqkv.T -> 16-head causal attention -> @ Wout.T.

Sharding: core c handles batch b=c//2, head-group g=c%2 (8 heads of 64).
Each core computes qkv for its (batch, head-group) slice, causal attention,
and a partial out-projection over its 512 columns of Wout's input dim.
Host sums the two partials per batch (the all-reduce of the hint).

Per-core layouts (host pre-transposes so every matmul contraction dim lands
on SBUF partitions):
  xT   [1024 d, 2048 t]      wqkT [1024 d, 1024 (q|k)e]
  wvT  [1024 d,  512 e]      woT  [ 512 e, 1024 f]
All matmuls run fp32r (1 cycle/col at N>=256).

Schedule: the attention inner loop is ACT(exp)-bound (~1.15us/pair-tile vs
~0.64us of PE work), so all other PE work is injected as filler between
attention pair-iterations to keep the PE dense and HAM warm:
  - chunk tc (tc<3) attention hosts the QKV production of chunk tc+1
  - chunk 2 also hosts chunk 0's out-projection; chunk 3 hosts chunks 1-2's
    (chunk 3 has no QKV filler), chunk 3's own runs at the tail
DMAs are ordered so the first QKV matmul's operands land first (wqk/x
interleaved, wv next, wo deferred to chunk 1).
"""

import sys

sys.path.insert(0, "/opt/trn_rl_repo")

import numpy as np

B, T, D, H = 4, 2048, 1024, 16
E = 512  # per-core head width (8 heads x 64)
ND = 8  # d chunks of 128
NTC = 4  # t chunks of 512
SCALE = 0.125  # 1/sqrt(64)

_NC_CACHE = {}


def build():
    if "nc" in _NC_CACHE:
        return _NC_CACHE["nc"]
    import concourse.bacc as bacc
    import concourse.mybir as mybir
    import concourse.tile as tile

    F32 = mybir.dt.float32
    F32R = mybir.dt.float32r
    BF16 = mybir.dt.bfloat16
    EXP = mybir.ActivationFunctionType.Exp

    nc = bacc.Bacc("TRN2", target_bir_lowering=False, debug=False, num_devices=8)
    xT = nc.declare_dram_parameter("xT", [D, T], BF16, isOutput=False)
    wqkT = nc.declare_dram_parameter("wqkT", [D, 2 * E], BF16, isOutput=False)
    wvT = nc.declare_dram_parameter("wvT", [D, E], BF16, isOutput=False)
    woT = nc.declare_dram_parameter("woT", [E, D], F32R, isOutput=False)
    z = nc.declare_dram_parameter("z", [T, D], F32, isOutput=True)

    with tile.TileContext(nc) as tc:
        with (
            tc.tile_pool(name="pw", bufs=8) as pw,
            tc.tile_pool(name="pwo", bufs=4) as pwo,
            tc.tile_pool(name="px", bufs=8) as px,
            tc.tile_pool(name="pkt", bufs=4) as pkt,
            tc.tile_pool(name="pqt", bufs=8) as pqt,
            tc.tile_pool(name="pv", bufs=16) as pv,
            tc.tile_pool(name="ppt", bufs=2) as ppt,
            tc.tile_pool(name="pr", bufs=1) as pr,
            tc.tile_pool(name="pysb", bufs=12) as pysb,
            tc.tile_pool(name="pzsb", bufs=2) as pzsb,
            tc.tile_pool(name="pone", bufs=1) as pone,
            tc.tile_pool(name="pst", bufs=2, space="PSUM") as pst,
            tc.tile_pool(name="pacc", bufs=2, space="PSUM") as pacc,
            tc.tile_pool(name="pyd", bufs=1, space="PSUM") as pyd,
        ):
            # ---- chunk-0 inputs, ordered for earliest first matmul:
            # wqk[dc]/x[dc] pairs interleaved, wv after, wo deferred.
            wqk = [None] * ND
            xs0 = [None] * ND
            for dc in range(ND):
                w_ = pw.tile([128, 2 * E], BF16, tag="wqk", name="wqk")
                nc.sync.dma_start(w_[:], wqkT[dc * 128 : (dc + 1) * 128, :])
                wqk[dc] = w_
                t_ = px.tile([128, 512], BF16, tag="x", name="xs")
                nc.sync.dma_start(t_[:], xT[dc * 128 : (dc + 1) * 128, 0:512])
                xs0[dc] = t_
            wv = []
            for dc in range(ND):
                t_ = pw.tile([128, E], BF16, tag="wv", name="wv")
                nc.sync.dma_start(t_[:], wvT[dc * 128 : (dc + 1) * 128, :])
                wv.append(t_)
            wo = [None] * 4  # loaded at chunk 1

            ones_f = pone.tile([128, 64], F32, tag="onef")
            nc.gpsimd.memset(ones_f[:], 1.0)

            # persistent K^T [e,t] tiles; pair m = heads 2m / 2m+1 at
            # partition rows 0:64 / 64:128
            kt = [
                pkt.tile([128, T], BF16, tag="kt", name=f"kt{i}")
                for i in range(4)
            ]
            vt = [None] * 16  # V tiles per 128-row t-block

            def emit_x_loads(tci):
                xs = []
                t0 = tci * 512
                for dc in range(ND):
                    t_ = px.tile([128, 512], BF16, tag="x", name="xs")
                    nc.sync.dma_start(
                        t_[:], xT[dc * 128 : (dc + 1) * 128, t0 : t0 + 512]
                    )
                    xs.append(t_)
                return xs

            def emit_qk_group(xs, m, tci):
                """m 0..3: Q chunk -> returns qt tile; 4..7: K chunk."""
                acc = pacc.tile([128, 512], F32, tag="acc", name="acc")
                for dc in range(ND):
                    nc.tensor.matmul(
                        acc[:],
                        wqk[dc][:, m * 128 : (m + 1) * 128],
                        xs[dc][:],
                        start=(dc == 0),
                        stop=(dc == ND - 1),
                    )
                if m < 4:
                    t_ = pqt.tile([128, 512], BF16, tag="qt", name="qt")
                    nc.vector.tensor_copy(t_[:], acc[:])
                    return t_
                t0 = tci * 512
                nc.vector.tensor_copy(kt[m - 4][:, t0 : t0 + 512], acc[:])
                return None

            def emit_v_group(xs, tci, ts):
                """V tile layout: [V_h(64) | ones(8)] per head, so AV
                lhsT [.., 72] slices put Y at PSUM rows 0:64 and the
                denominator at 64:72."""
                jb = 4 * tci + ts
                acc = pacc.tile([128, 512], F32, tag="acc", name="vacc")
                for dc in range(ND):
                    nc.tensor.matmul(
                        acc[:],
                        xs[dc][:, ts * 128 : (ts + 1) * 128],
                        wv[dc][:],
                        start=(dc == 0),
                        stop=(dc == ND - 1),
                    )
                t_ = pv.tile([128, 576], F32R, tag="v", name="vt")
                t4 = t_[:].rearrange("p (hh c) -> p hh c", hh=8)
                a4 = acc[:].rearrange("p (hh c) -> p hh c", hh=8)
                nc.vector.tensor_copy(t4[:, :, 0:64], a4[:])
                o4 = ones_f[:].rearrange("p (hh c) -> p hh c", hh=8)
                nc.vector.tensor_copy(t4[:, :, 64:72], o4[:])
                vt[jb] = t_

            def emit_wo_loads():
                for m in range(4):
                    t_ = pwo.tile([128, D], F32R, tag="wo", name="wo")
                    nc.sync.dma_start(t_[:], woT[m * 128 : (m + 1) * 128, :])
                    wo[m] = t_

            ysb_all = [[None] * 4 for _ in range(NTC)]

            def emit_op_block(c, ib, fh):
                """Out-projection for chunk c, row-block ib, col-half fh."""
                zp = pacc.tile([128, 512], F32, tag="acc", name="zp")
                for m in range(4):
                    nc.tensor.matmul(
                        zp[:],
                        ysb_all[c][m][:, ib * 128 : (ib + 1) * 128],
                        wo[m][:, fh * 512 : fh * 512 + 512],
                        start=(m == 0),
                        stop=(m == 3),
                    )
                zsb = pzsb.tile([128, 512], F32, tag="zsb", name="zsb")
                nc.vector.tensor_copy(zsb[:], zp[:])
                row = (4 * c + ib) * 128
                nc.sync.dma_start(
                    z[row : row + 128, fh * 512 : fh * 512 + 512], zsb[:]
                )

            # ---- prologue: chunk 0 QKV production
            qt_cur = [emit_qk_group(xs0, m, 0) for m in range(4)]
            for m in range(4, 8):
                emit_qk_group(xs0, m, 0)
            for ts in range(4):
                emit_v_group(xs0, 0, ts)

            for tc_i in range(NTC):
                xs_next = emit_x_loads(tc_i + 1) if tc_i + 1 < NTC else None
                if tc_i == 1:
                    emit_wo_loads()
                qt_next = [None] * 4 if xs_next is not None else None

                # filler queue: closures emitting independent PE work,
                # injected between attention pair-iterations
                fillers = []
                if xs_next is not None:

                    def mk_qk(mm):
                        def f():
                            r = emit_qk_group(xs_next, mm, tc_i + 1)
                            if mm < 4:
                                qt_next[mm] = r

                        return f

                    def mk_v(ts):
                        return lambda: emit_v_group(xs_next, tc_i + 1, ts)

                    for mm in range(4):
                        fillers.append(mk_qk(mm))
                        fillers.append(mk_qk(mm + 4))
                        fillers.append(mk_v(mm))
                if tc_i == 2:
                    for ib in range(4):
                        for fh in range(2):
                            fillers.append(
                                lambda ib=ib, fh=fh: emit_op_block(0, ib, fh)
                            )
                if tc_i == 3:
                    for c in (1, 2):
                        for ib in range(4):
                            for fh in range(2):
                                fillers.append(
                                    lambda c=c, ib=ib, fh=fh: emit_op_block(
                                        c, ib, fh
                                    )
                                )

                njb = 4 * tc_i + 4
                niters = 4 * njb
                nfill = len(fillers)
                npopped = 0
                it_g = 0
                # delay QKV fillers until their x-chunk DMAs have landed
                lead = 5 if xs_next is not None else 0

                def off_of(jb):
                    """Diagonal blocks only see queries >= 128*(jb-4*tc_i);
                    trim the streamed query range to the causal triangle."""
                    return 128 * (jb - 4 * tc_i) if jb >= 4 * tc_i else 0

                def emit_s(m, qtm, jb):
                    off = off_of(jb)
                    st = pst.tile([128, 1024], F32, tag="st", name="st")
                    for h in range(2):
                        nc.tensor.matmul(
                            st[:, h * 512 + off : h * 512 + 512],
                            kt[m][
                                h * 64 : h * 64 + 64,
                                jb * 128 : (jb + 1) * 128,
                            ],
                            qtm[h * 64 : h * 64 + 64, off:512],
                            start=True,
                            stop=True,
                        )
                    return st

                for m in range(4):
                    qtm = qt_cur[m]
                    ya = pyd.tile([72, 512], F32, tag="ya")
                    yb = pyd.tile([72, 512], F32, tag="yb")
                    st_next = emit_s(m, qtm, 0)
                    if it_g >= lead:
                        for _ in range(2):
                            if fillers:
                                fillers.pop(0)()
                                npopped += 1
                    for jb in range(njb):
                        off = off_of(jb)
                        st = st_next
                        pt = ppt.tile([128, 1024], F32R, tag="pt", name="pt")
                        if off:
                            stv = st[:].rearrange("p (h q) -> p h q", h=2)
                            ptv = pt[:].rearrange("p (h q) -> p h q", h=2)
                            nc.scalar.activation(
                                ptv[:, :, off:512],
                                stv[:, :, off:512],
                                EXP,
                                scale=SCALE,
                            )
                        else:
                            nc.scalar.activation(pt[:], st[:], EXP, scale=SCALE)
                        if jb + 1 < njb:
                            st_next = emit_s(m, qtm, jb + 1)
                        # PE filler while ACT runs the exp
                        it_g += 1
                        adj = max(0, it_g - lead)
                        while fillers and npopped < adj * nfill // (
                            niters - lead
                        ):
                            fillers.pop(0)()
                            npopped += 1
                        if jb >= 4 * tc_i:
                            for h in range(2):
                                half = pt[:, h * 512 + off : h * 512 + 512]
                                nc.gpsimd.affine_select(
                                    out=half,
                                    in_=half,
                                    compare_op=mybir.AluOpType.is_ge,
                                    fill=0.0,
                                    base=0,
                                    pattern=[[1, 512 - off]],
                                    channel_multiplier=-1,
                                )
                        first, last = (jb == 0), (jb == njb - 1)
                        nc.tensor.matmul(
                            ya[:, off:512],
                            vt[jb][:, m * 144 : m * 144 + 72],
                            pt[:, off:512],
                            start=first,
                            stop=last,
                        )
                        nc.tensor.matmul(
                            yb[:, off:512],
                            vt[jb][:, m * 144 + 72 : m * 144 + 144],
                            pt[:, 512 + off : 1024],
                            start=first,
                            stop=last,
                        )

                    # normalize. recip and partition_broadcast need
                    # partition-0 sources on HW (AP partition offsets are
                    # ignored by their ucode), so denom rows go 64 -> 0 via
                    # vector copies. Y and the denoms are pulled into SBUF
                    # first so the ya/yb PSUM banks free quickly (the next
                    # pair's first AV WAR-waits on them); the slow
                    # recip/broadcast/scale chain then runs off-path.
                    rci = pr.tile([128, 1024], F32, tag="rci", bufs=1)
                    nc.vector.tensor_copy(rci[0:1, 0:512], ya[64:65, :])
                    nc.vector.tensor_copy(rci[0:1, 512:1024], yb[64:65, :])
                    yta = pr.tile([64, 512], F32, tag="yta", bufs=1)
                    nc.vector.tensor_copy(yta[:], ya[0:64, :])
                    ytb = pr.tile([64, 512], F32, tag="ytb", bufs=1)
                    nc.vector.tensor_copy(ytb[:], yb[0:64, :])
                    rco = pr.tile([128, 1024], F32, tag="rco", bufs=1)
                    nc.vector.reciprocal_approx_fast(rco[0:1, :], rci[0:1, :])
                    rb = pr.tile([128, 1024], F32, tag="rb", bufs=1)
                    nc.gpsimd.partition_broadcast(rb[0:64, :], rco[0:1, :])
                    ytmp = pr.tile([128, 512], F32R, tag="ytmp", bufs=1)
                    nc.vector.tensor_mul(
                        ytmp[0:64, :], ytb[:], rb[0:64, 512:1024]
                    )
                    ysb = pysb.tile([128, 512], F32R, tag="ysb", name="ysb")
                    nc.vector.tensor_mul(
                        ysb[0:64, :], yta[:], rb[0:64, 0:512]
                    )
                    nc.sync.dma_start(ysb[64:128, :], ytmp[0:64, :])
                    ysb_all[tc_i][m] = ysb

                # drain any leftover fillers
                while fillers:
                    fillers.pop(0)()

                if tc_i == 3:
                    for ib in range(4):
                        for fh in range(2):
                            emit_op_block(3, ib, fh)

                qt_cur = qt_next

    nc.finalize()
    _NC_CACHE["nc"] = nc
    return nc


def _in_maps(x, Wqkv, Wout):
    import ml_dtypes

    BF = ml_dtypes.bfloat16
    x = np.ascontiguousarray(np.asarray(x, dtype=np.float32))
    Wqkv = np.ascontiguousarray(np.asarray(Wqkv, dtype=np.float32))
    Wout = np.ascontiguousarray(np.asarray(Wout, dtype=np.float32))
    xTs = [np.ascontiguousarray(x[b].T.astype(BF)) for b in range(B)]
    maps = []
    for c in range(8):
        b, g = divmod(c, 2)
        qrows = Wqkv[E * g : E * g + E]
        krows = Wqkv[D + E * g : D + E * g + E]
        vrows = Wqkv[2 * D + E * g : 2 * D + E * g + E]
        maps.append(
            {
                "xT": xTs[b],
                "wqkT": np.ascontiguousarray(
                    np.concatenate([qrows, krows], axis=0).T.astype(BF)
                ),
                "wvT": np.ascontiguousarray(vrows.T.astype(BF)),
                "woT": np.ascontiguousarray(Wout[:, E * g : E * g + E].T),
            }
        )
    return maps


def _run(x, Wqkv, Wout, trace=False):
    from concourse.bass_utils import run_bass_kernel_spmd

    nc = build()
    res = run_bass_kernel_spmd(
        nc, _in_maps(x, Wqkv, Wout), core_ids=list(range(8)), trace=trace
    )
    out = np.empty((B, T, D), dtype=np.float32)
    for b in range(B):
        out[b] = res.results[2 * b]["z"] + res.results[2 * b + 1]["z"]
    return out, res


def kernel(x, Wqkv, Wout):
    out, _ = _run(x, Wqkv, Wout, trace=False)
    return out
